# revision 1
# baseline (speedup 1.0000x reference)
"""Trainium2 Bass kernel for nn_ChamferDistanceL2.

Math notes (exact reformulation of the reference):
  probs = softmax(logits) over V; the chamfer "y" cloud is one-hot rows of
  targets (masked), so the pairwise squared distances collapse to
      d2[b,i,j] = xs_i + mask_j - 2*mask_i*mask_j*probs[b,i,t_j]
  with xs_i = mask_i * sum_{v>=1} probs[b,i,v]^2.  Everything the device
  needs from the full [B,S,V] logits is:
      s_i  = sum_v exp(l)       (ACT exp pass, accum)
      q_i  = sum_v exp(l)^2     (DVE bn_stats moments over the exp tile,
                                 q = V*(var + mean^2); ACT exp(2l) for the
                                 last batches to shorten the DVE tail)
      e0_i = exp(l[...,0]), and the gathered raw logits l[b,i,t_j].
  The gather indices/masks are pure functions of the tiny `targets` input,
  so the host precomputes them.  The device streams the 16MB logits shard,
  does all the exp/softmax-stat/chamfer-min work, and returns per-row s and
  the two per-batch min columns; the final [B,S]-level BCE scalars and means
  are finished on the host (0.02% of the FLOPs).
"""

import os
import sys

sys.path.insert(0, "/opt/trn_rl_repo")

import numpy as np

B, S, V = 64, 128, 4096
M = 8                 # NeuronCores (data-parallel over batch)
BC = B // M           # batch elements per core
R = BC * S            # rows per core
SW = S + 1            # gather width (128 targets + eos col)
EOS, PAD, EPS = 0, 4096, 1e-8
NEG = np.float32(-1e30)

_CACHE = {}


def _build_nc(reps=1, q_mode="mix", k_act=1, big_bufs=5, scr_bufs=5,
              dma_only=False, split_last=4, act_q=(1, 3),
              tail_perm=(0, 2, 1, 3)):
    # split_last: how many of the trailing batches get half-split DMAs/exps
    # act_q: extra batch indices whose q runs as an ACT exp(2l) pass
    """q_mode: 'act' = q via exp(2l) ACT pass; 'bn' = q via DVE bn_stats;
    'mix' = bn for all but the LAST k_act batches (ACT is free at the end,
    DVE is the tail bottleneck)."""
    import concourse.bacc as bacc
    import concourse.mybir as mybir
    from concourse.tile import TileContext
    from concourse.masks import make_identity

    f32 = mybir.dt.float32
    bf16 = mybir.dt.bfloat16
    A = mybir.AluOpType
    AF = mybir.ActivationFunctionType
    X = mybir.AxisListType.X

    nc = bacc.Bacc()
    lgt = nc.dram_tensor("lgt", [R, V], f32, kind="ExternalInput")
    # lgep: gathered raw logits packed [128, BC*SW]; col b*SW+S is l0
    lgep = nc.dram_tensor("lgep", [128, BC * SW], f32, kind="ExternalInput")
    mcolp = nc.dram_tensor("mcolp", [128, BC], f32, kind="ExternalInput")
    mrowp = nc.dram_tensor("mrowp", [1, BC * S], f32, kind="ExternalInput")
    # out columns: 0..BC-1 = s rows, BC..2BC-1 = min_j d2 (per i),
    # 2BC..3BC-1 = min_i d2 (per j)
    out = nc.dram_tensor("out", [128, 3 * BC], f32, kind="ExternalOutput")

    with TileContext(nc) as tc:
        with (
            tc.tile_pool(name="big", bufs=big_bufs) as bigp,
            tc.tile_pool(name="scr", bufs=scr_bufs) as scrp,
            tc.tile_pool(name="aux", bufs=2) as auxp,
            tc.tile_pool(name="sm", bufs=3) as smp,
            tc.tile_pool(name="keep", bufs=1) as keepp,
            tc.tile_pool(name="ps", bufs=2, space="PSUM") as psp,
            tc.tile_pool(name="psb", bufs=2, space="PSUM") as psbp,
        ):
            out_sb = keepp.tile([128, 3 * BC], f32, tag="outsb")
            if dma_only:
                nc.vector.memset(out_sb[:], 0.0)

            # reps>1 repeats the computation for marginal-time benchmarking
            for _rep in range(reps):
                # ---- issue every input DMA up front (13 instructions);
                # small aux first so they don't queue behind the 16MB ----
                t_lgep = auxp.tile([128, BC * SW], f32, tag="lgep")
                nc.sync.dma_start(out=t_lgep[:, :], in_=lgep[:, :])
                t_mcol = auxp.tile([128, BC], f32, tag="mcolp")
                nc.sync.dma_start(out=t_mcol[:, :], in_=mcolp[:, :])
                t_mrow = auxp.tile([1, BC * S], f32, tag="mrowp")
                nc.sync.dma_start(out=t_mrow[:, :], in_=mrowp[:, :])
                t_lgts = []
                NSPL = 2
                H = V // NSPL
                dma_q = []
                for b in range(BC):
                    t_lgt = bigp.tile([128, V], f32, tag="lgt")
                    rows = slice(b * 128, (b + 1) * 128)
                    if split_last and b >= BC - split_last:
                        for h in range(NSPL):
                            cs = slice(h * H, (h + 1) * H)
                            dma_q.append((t_lgt, cs, rows))
                    else:
                        dma_q.append((t_lgt, slice(0, V), rows))
                    t_lgts.append(t_lgt)
                # reorder trailing half-DMA slices (tail_perm indexes the
                # last len(tail_perm) entries of the natural order)
                if tail_perm:
                    n = len(tail_perm)
                    tail = dma_q[-n:]
                    dma_q[-n:] = [tail[i] for i in tail_perm]
                for t_lgt_, cs_, rows_ in dma_q:
                    nc.sync.dma_start(out=t_lgt_[:, cs_], in_=lgt[rows_, cs_])

                if _rep == 0:
                    ones1 = keepp.tile([1, S], f32, tag="ones1")
                    nc.vector.memset(ones1[:], 1.0)
                    identp = keepp.tile([128, 128], f32, tag="identp")
                    make_identity(nc, identp[:])

                if dma_only:
                    nc.sync.dma_start(out=out[:, :], in_=out_sb[:, :])
                    continue

                # PE: broadcast all 8 mask_j rows to [128, BC*S] in PSUM via
                # two 512-wide K=1 outer products (ones^T x mrow)
                mjb_ps = []
                for h in range(2):
                    ps = psbp.tile([128, BC * S // 2], f32, tag=f"mjb{h}")
                    nc.tensor.matmul(
                        ps[:], lhsT=ones1[:],
                        rhs=t_mrow[:, h * (BC * S // 2) : (h + 1) * (BC * S // 2)],
                        start=True, stop=True,
                    )
                    mjb_ps.append(ps)

                # one exp over all gathered columns (8*129 wide)
                eg_all = auxp.tile([128, BC * SW], f32, tag="egall")
                nc.scalar.activation(eg_all[:], t_lgep[:], AF.Exp)

                for b in range(BC):
                    t_lgt = t_lgts[b]
                    mh = t_mcol[:, b : b + 1]

                    # ACT: exp pass over the [128, V] tile; accum -> s column
                    scr1 = scrp.tile([128, V], f32, tag="scr")
                    q = smp.tile([128, 1], f32, tag="q")
                    if split_last and b >= BC - split_last:
                        # per-slice s-exp (and for the very last batch also
                        # q-exp) so each slice's work starts as soon as its
                        # DMA lands
                        last = b == BC - 1
                        if last:
                            scr2 = scrp.tile([128, V], f32, tag="scr")
                            qparts = smp.tile([128, NSPL], f32, tag="qparts")
                        sparts = smp.tile([128, NSPL], f32, tag="sparts")
                        for h in range(NSPL):
                            cs = slice(h * H, (h + 1) * H)
                            nc.scalar.activation(
                                scr1[:, cs], t_lgt[:, cs], AF.Exp,
                                accum_out=sparts[:, h : h + 1],
                            )
                            if last:
                                nc.scalar.activation(
                                    scr2[:, cs], t_lgt[:, cs], AF.Exp,
                                    scale=2.0,
                                    accum_out=qparts[:, h : h + 1],
                                )
                        nc.vector.tensor_reduce(
                            out_sb[:, b : b + 1], sparts[:], axis=X, op=A.add
                        )
                        if last:
                            nc.vector.tensor_reduce(
                                q[:], qparts[:], axis=X, op=A.add
                            )
                    else:
                        nc.scalar.activation(
                            scr1[:], t_lgt[:], AF.Exp,
                            accum_out=out_sb[:, b : b + 1],
                        )
                    use_dve = (
                        (not (split_last and b == BC - 1))
                        and b not in act_q
                        and (
                            q_mode in ("bn", "ttr")
                            or (q_mode in ("mix", "mixttr") and b < BC - k_act)
                        )
                    )
                    use_ttr = q_mode in ("ttr", "mixttr")
                    if use_dve and use_ttr:
                        # q = sum(E*E) in one fused DVE pass (broadcast dummy
                        # out, the kernels/qr.py pattern)
                        dumq = smp.tile([128, 1], f32, tag="dumq")
                        nc.vector.tensor_tensor_reduce(
                            out=dumq[:].broadcast_to([128, V]), in0=scr1[:],
                            in1=scr1[:], scale=1.0, scalar=0.0,
                            op0=A.mult, op1=A.add, accum_out=q[:],
                        )
                    elif use_dve:
                        # q = sum(E^2) from bn_stats moments (tile_groupnorm
                        # pattern): 8 x 512-wide stats + aggregate
                        NSUB = V // 512
                        er = scr1[:].rearrange("p (n s) -> p n s", s=512)
                        stats = smp.tile([128, NSUB, 6], f32, tag="bnst")
                        for sg in range(NSUB):
                            nc.vector.bn_stats(
                                out=stats[:, sg, :], in_=er[:, sg, :]
                            )
                        mv = smp.tile([128, 2], f32, tag="bnmv")
                        nc.vector.bn_aggr(out=mv[:], in_=stats[:])
                        m2 = smp.tile([128, 1], f32, tag="bnm2")
                        nc.vector.tensor_mul(m2[:], mv[:, 0:1], mv[:, 0:1])
                        vpm = smp.tile([128, 1], f32, tag="bnvpm")
                        nc.vector.tensor_add(vpm[:], mv[:, 1:2], m2[:])
                        nc.vector.tensor_scalar(
                            q[:], vpm[:], float(V), None, A.mult
                        )
                    elif not (split_last and b == BC - 1):
                        scr2 = scrp.tile([128, V], f32, tag="scr")
                        nc.scalar.activation(
                            scr2[:], t_lgt[:], AF.Exp, scale=2.0,
                            accum_out=q[:],
                        )

                    eg = eg_all[:, b * SW : b * SW + S]
                    e0 = eg_all[:, b * SW + S : b * SW + S + 1]
                    # DVE: per-row softmax stats (fused two-scalar forms;
                    # signs folded so d2a ends up identical)
                    rs = smp.tile([128, 1], f32, tag="rs")
                    nc.vector.reciprocal(rs[:], out_sb[:, b : b + 1])
                    qm = smp.tile([128, 1], f32, tag="qm")
                    nc.vector.tensor_scalar(       # e0^2 - q  (= -(q-e0^2))
                        qm[:], e0, e0, q[:], A.mult, A.subtract
                    )
                    rs2m = smp.tile([128, 1], f32, tag="rs2m")
                    nc.vector.tensor_scalar(       # rs^2 * mh
                        rs2m[:], rs[:], rs[:], mh, A.mult, A.mult
                    )
                    xs = smp.tile([128, 1], f32, tag="xs")
                    nc.vector.tensor_mul(xs[:], qm[:], rs2m[:])   # = -xs_true
                    m2rsm = smp.tile([128, 1], f32, tag="m2rsm")
                    nc.vector.tensor_scalar(       # -2 * rs * mh
                        m2rsm[:], rs[:], -2.0, mh, A.mult, A.mult
                    )

                    # DVE: chamfer distance matrix and its two mins
                    mjb = mjb_ps[b // 4][:, (b % 4) * S : (b % 4 + 1) * S]
                    d2a = smp.tile([128, S], f32, tag="d2a")
                    nc.vector.tensor_scalar(       # eg*(-2 rs mh) - (-xs_true)
                        d2a[:], eg, m2rsm[:], xs[:], A.mult, A.subtract
                    )
                    # row mins need the +mask_j term elementwise
                    d2 = smp.tile([128, S], f32, tag="d2")
                    nc.vector.tensor_add(d2[:], d2a[:], mjb)
                    nc.vector.tensor_reduce(
                        out_sb[:, BC + b : BC + b + 1], d2[:], axis=X, op=A.min
                    )
                    # column mins: transpose d2a directly (doesn't wait for
                    # the mask add); in the transposed view mask_j is a
                    # per-partition constant, so add it after the min
                    # (exact: min commutes with a row-constant add)
                    pt = psp.tile([128, 128], f32, tag="pt")
                    nc.tensor.transpose(pt[:], d2a[:], identp[:])
                    ptm = smp.tile([128, 1], f32, tag="ptm")
                    nc.vector.tensor_reduce(ptm[:], pt[:], axis=X, op=A.min)
                    nc.vector.tensor_add(
                        out_sb[:, 2 * BC + b : 2 * BC + b + 1], ptm[:],
                        t_mcol[:, b : b + 1],
                    )

                nc.sync.dma_start(out=out[:, :], in_=out_sb[:, :])

    nc.compile()
    return nc


def _get_nc():
    if "nc" not in _CACHE:
        _CACHE["nc"] = _build_nc()
    return _CACHE["nc"]


def _prep(logits, targets):
    """Host-side prep: masks, counts, gathered raw logits (all derived from
    the tiny `targets` tensor + a 4MB fancy-index into logits)."""
    logits = np.ascontiguousarray(np.asarray(logits, dtype=np.float32))
    t = np.asarray(targets).astype(np.int64)
    mh = ((t != PAD) & (t != EOS)).astype(np.float32)   # eos_head
    tclip = np.minimum(t, V - 1)
    lg = np.take_along_axis(
        logits, np.broadcast_to(tclip[:, None, :], (B, S, S)), axis=2
    )
    lgm = np.where(mh[:, None, :] > 0, lg, NEG)
    lge = np.concatenate([lgm, logits[:, :, 0:1]], axis=2)       # [B,S,SW]
    return logits, lge, mh, t


def _in_maps(logits, lge, mh):
    maps = []
    for c in range(M):
        bs = slice(c * BC, (c + 1) * BC)
        # pack gathered logits as [128, BC*SW] (batch-major columns)
        lgep = np.ascontiguousarray(
            lge[bs].transpose(1, 0, 2).reshape(S, BC * SW)
        )
        maps.append(
            {
                "lgt": np.ascontiguousarray(logits[bs].reshape(R, V)),
                "lgep": lgep,
                "mcolp": np.ascontiguousarray(mh[bs].T),     # [128, BC]
                "mrowp": np.ascontiguousarray(mh[bs].reshape(1, BC * S)),
            }
        )
    return maps


def _combine(outs, logits, mh, t):
    """outs: [M][128, 3*BC] -> final [2] float32.  Finishes the reduction
    layer on the host: chamfer means from the device min columns, BCE from
    the device softmax denominators."""
    f = np.float32
    o = np.stack([np.asarray(x) for x in outs])        # [M, 128, 3*BC]
    s = o[:, :, 0:BC].transpose(0, 2, 1).reshape(B, S).astype(f)
    dmin_i = o[:, :, BC : 2 * BC].transpose(0, 2, 1).reshape(B, S)
    dmin_j = o[:, :, 2 * BC : 3 * BC].transpose(0, 2, 1).reshape(B, S)
    label = np.mean((dmin_i.sum(1) + dmin_j.sum(1)) / S)

    # BCE (host, f32, matching the reference's formulas)
    l0 = logits[:, :, 0].astype(f)
    e0 = np.exp(l0).astype(f)
    rs = (1.0 / s).astype(f)
    p0 = (e0 * rs).astype(f)
    logp = np.maximum((l0 - np.log(s).astype(f)).astype(f), f(-100.0))
    lom = np.maximum(np.log1p(-p0).astype(f), f(-100.0))
    et = (mh == 0)                                     # eos_target
    bce = np.where(et, -logp, -lom).astype(f)
    ep = (t == EOS).astype(f)
    eh = mh
    cep, ceh = ep.sum(1), eh.sum(1)
    eos = np.mean(
        0.5 * (bce * ep).sum(1) / (cep + EPS)
        + 0.5 * (bce * eh).sum(1) / (ceh + EPS)
    )
    return np.stack([label, eos]).astype(f)


def kernel(logits, targets):
    logits, lge, mh, t = _prep(logits, targets)
    maps = _in_maps(logits, lge, mh)
    nc = _get_nc()

    if os.environ.get("KMODE") == "sim":
        from concourse import bass_interp

        outs = []
        for c in range(M):
            sim = bass_interp.CoreSim(nc)
            for k, v in maps[c].items():
                sim.tensor(k)[:] = v
            sim.simulate()
            outs.append(np.array(sim.tensor("out")))
    else:
        import time

        from concourse.bass_utils import run_bass_kernel_spmd

        # the axon terminal occasionally reports a transient mesh desync;
        # a short backoff and retry recovers it
        last_err = None
        for attempt in range(3):
            try:
                res = run_bass_kernel_spmd(nc, maps, list(range(M)))
                break
            except Exception as e:  # noqa: BLE001
                last_err = e
                time.sleep(30 * (attempt + 1))
        else:
            raise last_err
        outs = [res.results[c]["out"] for c in range(M)]

    return _combine(outs, logits, mh, t)



# revision 4
# speedup vs baseline: 1.1831x; 1.1831x over previous
"""Trainium2 Bass kernel for nn_ChamferDistanceL2.

Math notes (exact reformulation of the reference):
  probs = softmax(logits) over V; the chamfer "y" cloud is one-hot rows of
  targets (masked), so the pairwise squared distances collapse to
      d2[b,i,j] = xs_i + mask_j - 2*mask_i*probs[b,i,t_j]*mask_j
  with xs_i = mask_i * sum_{v>=1} probs[b,i,v]^2.  Everything the device
  needs from the full [B,S,V] logits is:
      s_i  = sum_v exp(l)       (ACT exp pass with accumulator)
      q_i  = sum_v exp(l)^2     (DVE tensor_tensor_reduce / GPSIMD
                                 scalar_tensor_tensor over the exp tile)
  plus the gathered values exp(l[b,i,t_j]) and exp(l[b,i,0]), which are
  pure functions of the tiny `targets` tensor and a 4MB fancy-index of
  logits, so the host precomputes them.  The device streams the logits
  shard (bf16, 8MB/core), does the exp / moment / chamfer-min work, and
  returns per-row s and the two per-batch min columns; the final
  [B,S]-level BCE scalars and means are finished on the host (0.02% of
  the FLOPs).

Performance structure (cost-model driven):
  - logits are downcast to bf16 on the host: halves HBM traffic; the DMA
    device otherwise dominates (f32 stream = 46us/core > ACT exp 31us).
  - ACT does exactly one exp pass per tile (8 tiles of [128,4096]) with
    the free accumulator producing s.  This ~31us is the compute floor.
  - q is balanced across the otherwise-idle engines: GPSIMD
    (scalar_tensor_tensor accum) takes the early tiles, DVE
    (tensor_tensor_reduce) the late ones, all hidden under ACT.
  - first/last tiles are DMA'd and exp'd in chunks to cut the startup
    lag and the post-exp tail.
  - row-min is fused into one TTR (op0=add with the mask row, op1=min
    accumulate); col-min transposes d2a on the idle PE and the DVE
    reduce for batch b is deferred behind batch b+1's work to hide the
    PE latency.
"""

import os
import sys

sys.path.insert(0, "/opt/trn_rl_repo")

import numpy as np

B, S, V = 64, 128, 4096
M = 8                 # NeuronCores (data-parallel over batch)
BC = B // M           # batch elements per core
R = BC * S            # rows per core
EOS, PAD, EPS = 0, 4096, 1e-8

_CACHE = {}

# q-engine per tile: 'P' = GPSIMD scalar_tensor_tensor, 'V' = DVE TTR.
Q_ENG = ("P", "P", "P", "V", "V", "P", "V", "V")
CH0 = 4               # DMA/exp chunks for the first tile (startup lag)
CH7 = 4               # chunks for the last tile (tail)


def _build_nc(q_eng=Q_ENG, ch0=CH0, ch7=CH7):
    import concourse.bacc as bacc
    import concourse.mybir as mybir
    from concourse.tile import TileContext
    from concourse.masks import make_identity

    f32 = mybir.dt.float32
    bf16 = mybir.dt.bfloat16
    A = mybir.AluOpType
    AF = mybir.ActivationFunctionType
    X = mybir.AxisListType.X

    nc = bacc.Bacc()
    lgt = nc.dram_tensor("lgt", [R, V], bf16, kind="ExternalInput")
    # host-exp'd gathered logits, packed [128, BC*S] batch-major
    egep = nc.dram_tensor("egep", [128, BC * S], bf16, kind="ExternalInput")
    # mask row replicated to all partitions (row-min TTR second operand)
    mrowb = nc.dram_tensor("mrowb", [128, BC * S], bf16, kind="ExternalInput")
    e0p = nc.dram_tensor("e0p", [128, BC], f32, kind="ExternalInput")
    mcolp = nc.dram_tensor("mcolp", [128, BC], f32, kind="ExternalInput")
    # out columns: 0..BC-1 = s rows, BC..2BC-1 = min_j d2 (per i),
    # 2BC..3BC-1 = min_i d2 (per j)
    out = nc.dram_tensor("out", [128, 3 * BC], f32, kind="ExternalOutput")

    chunks = {0: ch0, BC - 1: ch7}

    with TileContext(nc) as tc:
        with (
            tc.tile_pool(name="lg", bufs=BC) as lgp,
            tc.tile_pool(name="ex", bufs=BC) as exp_,
            tc.tile_pool(name="aux", bufs=1) as auxp,
            tc.tile_pool(name="sm", bufs=4) as smp,
            tc.tile_pool(name="keep", bufs=1) as keepp,
            tc.tile_pool(name="ps", bufs=3, space="PSUM") as psp,
        ):
            out_sb = keepp.tile([128, 3 * BC], f32, tag="outsb")

            # ---- input DMAs; first tile in chunks so exp starts early ----
            t_lgts = [
                lgp.tile([128, V], bf16, tag="lgt", name=f"lgt{b}")
                for b in range(BC)
            ]
            dma_q = []
            for b in range(BC):
                rows = slice(b * 128, (b + 1) * 128)
                n = chunks.get(b, 1)
                h = V // n
                for c in range(n):
                    cs = slice(c * h, (c + 1) * h)
                    dma_q.append((t_lgts[b], cs, rows))
            t_egep = auxp.tile([128, BC * S], bf16, tag="egep")
            t_mrow = auxp.tile([128, BC * S], bf16, tag="mrowb")
            t_e0 = auxp.tile([128, BC], f32, tag="e0p")
            t_mcol = auxp.tile([128, BC], f32, tag="mcolp")
            aux_q = [
                (t_egep, egep), (t_mrow, mrowb), (t_e0, e0p), (t_mcol, mcolp)
            ]
            # big tile 0 chunks + tile 1 first, then aux, then the rest
            for t, cs, rows in dma_q[: ch0 + 1]:
                nc.sync.dma_start(out=t[:, cs], in_=lgt[rows, cs])
            for t, src in aux_q:
                nc.sync.dma_start(out=t[:, :], in_=src[:, :])
            for t, cs, rows in dma_q[ch0 + 1 :]:
                nc.sync.dma_start(out=t[:, cs], in_=lgt[rows, cs])

            identp = keepp.tile([128, 128], f32, tag="identp")
            make_identity(nc, identp[:])

            # ---- ACT stream: one exp pass per tile, accum -> s ----
            sparts = {}
            t_Es = [
                exp_.tile([128, V], bf16, tag="E", name=f"E{b}")
                for b in range(BC)
            ]
            for b in range(BC):
                t_es = t_Es[b]
                n = chunks.get(b, 1)
                if n == 1:
                    nc.scalar.activation(
                        t_es[:], t_lgts[b][:], AF.Exp,
                        accum_out=out_sb[:, b : b + 1],
                    )
                else:
                    h = V // n
                    sp = smp.tile([128, n], f32, tag=f"sparts{b}")
                    sparts[b] = sp
                    for c in range(n):
                        cs = slice(c * h, (c + 1) * h)
                        nc.scalar.activation(
                            t_es[:, cs], t_lgts[b][:, cs], AF.Exp,
                            accum_out=sp[:, c : c + 1],
                        )

            # ---- Pool stream: q for its assigned tiles ----
            qcol = keepp.tile([128, BC], f32, tag="qcol")
            for b in range(BC):
                if q_eng[b] != "P":
                    continue
                dum = smp.tile([128, 1], bf16, tag="dumP")
                nc.gpsimd.scalar_tensor_tensor(
                    out=dum[:].broadcast_to([128, V]),
                    in0=t_Es[b][:], scalar=1.0, in1=t_Es[b][:],
                    op0=A.mult, op1=A.mult, accum_out=qcol[:, b : b + 1],
                )

            # ---- DVE stream (+ PE transposes), in readiness order ----
            st = {}   # per-batch s-derived stats tiles

            def s_stats(b):
                """rs, m2rs, rs2m, e2 for batch b (dep: s_b only)."""
                if b in sparts:
                    nc.vector.tensor_reduce(
                        out_sb[:, b : b + 1], sparts[b][:], axis=X, op=A.add
                    )
                rs = smp.tile([128, 1], f32, tag="rs")
                nc.vector.reciprocal(rs[:], out_sb[:, b : b + 1])
                tm = smp.tile([128, 1], f32, tag="tm")
                nc.vector.tensor_mul(tm[:], rs[:], t_mcol[:, b : b + 1])
                m2rs = smp.tile([128, 1], f32, tag="m2rs")
                nc.vector.tensor_scalar(m2rs[:], tm[:], -2.0, None, A.mult)
                rs2m = smp.tile([128, 1], f32, tag="rs2m")
                nc.vector.tensor_mul(rs2m[:], tm[:], rs[:])
                e2 = smp.tile([128, 1], f32, tag="e2")
                nc.vector.tensor_mul(
                    e2[:], t_e0[:, b : b + 1], t_e0[:, b : b + 1]
                )
                st[b] = (m2rs, rs2m, e2)

            def q_dve(b):
                dum = smp.tile([128, 1], f32, tag="dumV")
                nc.vector.tensor_tensor_reduce(
                    out=dum[:].broadcast_to([128, V]),
                    in0=t_Es[b][:], in1=t_Es[b][:],
                    scale=1.0, scalar=0.0, op0=A.mult, op1=A.add,
                    accum_out=qcol[:, b : b + 1],
                )

            pts = {}

            def cham_a(b):
                """qm, xsn, d2a, fused row-min, PE transpose (dep: q_b)."""
                m2rs, rs2m, e2 = st[b]
                qm = smp.tile([128, 1], f32, tag="qm")
                nc.vector.tensor_sub(qm[:], e2[:], qcol[:, b : b + 1])
                xsn = smp.tile([128, 1], f32, tag="xsn")
                nc.vector.tensor_mul(xsn[:], qm[:], rs2m[:])
                d2a = smp.tile([128, S], f32, tag="d2a")
                nc.vector.tensor_scalar(
                    d2a[:], t_egep[:, b * S : (b + 1) * S], m2rs[:], xsn[:],
                    A.mult, A.subtract,
                )
                dum = smp.tile([128, 1], f32, tag="dumR")
                nc.vector.tensor_tensor_reduce(
                    out=dum[:].broadcast_to([128, S]),
                    in0=d2a[:], in1=t_mrow[:, b * S : (b + 1) * S],
                    scale=1.0, scalar=1e30, op0=A.add, op1=A.min,
                    accum_out=out_sb[:, BC + b : BC + b + 1],
                )
                pt = psp.tile([128, 128], f32, tag="pt")
                nc.tensor.transpose(pt[:], d2a[:], identp[:])
                pts[b] = pt

            def cham_b(b):
                """col-min from the transposed d2a (dep: PE transpose b)."""
                ptm = smp.tile([128, 1], f32, tag="ptm")
                nc.vector.tensor_reduce(ptm[:], pts[b][:], axis=X, op=A.min)
                nc.vector.tensor_add(
                    out_sb[:, 2 * BC + b : 2 * BC + b + 1], ptm[:],
                    t_mcol[:, b : b + 1],
                )

            # DVE q-chunks for the last tile interleave with its s-stats
            def q_dve_chunks(b, n):
                h = V // n
                qp = smp.tile([128, n], f32, tag="qparts")
                for c in range(n - 1):
                    cs = slice(c * h, (c + 1) * h)
                    dum = smp.tile([128, 1], f32, tag="dumV")
                    nc.vector.tensor_tensor_reduce(
                        out=dum[:].broadcast_to([128, h]),
                        in0=t_Es[b][:, cs], in1=t_Es[b][:, cs],
                        scale=1.0, scalar=0.0, op0=A.mult, op1=A.add,
                        accum_out=qp[:, c : c + 1],
                    )
                s_stats(b)            # overlaps the last q chunk's deps
                cs = slice((n - 1) * h, n * h)
                dum = smp.tile([128, 1], f32, tag="dumV")
                nc.vector.tensor_tensor_reduce(
                    out=dum[:].broadcast_to([128, h]),
                    in0=t_Es[b][:, cs], in1=t_Es[b][:, cs],
                    scale=1.0, scalar=0.0, op0=A.mult, op1=A.add,
                    accum_out=qp[:, n - 1 : n],
                )
                nc.vector.tensor_reduce(
                    qcol[:, b : b + 1], qp[:], axis=X, op=A.add
                )

            # readiness-ordered DVE emission; col-min (cham_b) deferred one
            # batch so the PE transpose latency is hidden.
            s_stats(0)
            s_stats(1)
            s_stats(2)
            cham_a(0)                       # dep: pool q0
            q_dve(3)
            s_stats(3)
            cham_a(1)
            cham_b(0)
            q_dve(4)
            s_stats(4)
            cham_a(2)
            cham_b(1)
            cham_a(3)
            cham_b(2)
            cham_a(4)
            cham_b(3)
            s_stats(5)
            cham_a(5)                       # dep: pool q5
            cham_b(4)
            q_dve(6)
            s_stats(6)
            cham_a(6)
            cham_b(5)
            q_dve_chunks(7, ch7)            # includes s_stats(7)
            cham_a(7)
            cham_b(6)
            cham_b(7)

            nc.sync.dma_start(out=out[:, :], in_=out_sb[:, :])

    nc.compile()
    return nc


def _get_nc():
    if "nc" not in _CACHE:
        _CACHE["nc"] = _build_nc()
    return _CACHE["nc"]


def _prep(logits, targets):
    """Host-side prep: masks, counts, exp of the gathered raw logits (all
    derived from the tiny `targets` tensor + a 4MB fancy-index into
    logits), and the bf16 downcast of the big streamed tensor."""
    import ml_dtypes

    logits = np.ascontiguousarray(np.asarray(logits, dtype=np.float32))
    t = np.asarray(targets).astype(np.int64)
    mh = ((t != PAD) & (t != EOS)).astype(np.float32)   # eos_head
    tclip = np.minimum(t, V - 1)
    lg = np.take_along_axis(
        logits, np.broadcast_to(tclip[:, None, :], (B, S, S)), axis=2
    )
    eg = np.exp(lg, dtype=np.float32) * (mh[:, None, :] > 0)
    lgt16 = logits.astype(ml_dtypes.bfloat16)
    return logits, lgt16, eg, mh, t


def _in_maps(lgt16, eg, mh, logits):
    import ml_dtypes

    bf16 = ml_dtypes.bfloat16
    e0 = np.exp(logits[:, :, 0], dtype=np.float32)      # [B, S]
    maps = []
    for c in range(M):
        bs = slice(c * BC, (c + 1) * BC)
        # pack gathered exps as [128, BC*S] (batch-major columns)
        egep = np.ascontiguousarray(
            eg[bs].transpose(1, 0, 2).reshape(S, BC * S).astype(bf16)
        )
        mrowb = np.ascontiguousarray(
            np.broadcast_to(
                mh[bs].reshape(1, BC * S).astype(bf16), (S, BC * S)
            )
        )
        maps.append(
            {
                "lgt": np.ascontiguousarray(lgt16[bs].reshape(R, V)),
                "egep": egep,
                "mrowb": mrowb,
                "e0p": np.ascontiguousarray(e0[bs].T),       # [128, BC]
                "mcolp": np.ascontiguousarray(mh[bs].T),     # [128, BC]
            }
        )
    return maps


def _combine(outs, logits, mh, t):
    """outs: [M][128, 3*BC] -> final [2] float32.  Finishes the reduction
    layer on the host: chamfer means from the device min columns, BCE from
    the device softmax denominators."""
    f = np.float32
    o = np.stack([np.asarray(x) for x in outs])        # [M, 128, 3*BC]
    s = o[:, :, 0:BC].transpose(0, 2, 1).reshape(B, S).astype(f)
    dmin_i = o[:, :, BC : 2 * BC].transpose(0, 2, 1).reshape(B, S)
    dmin_j = o[:, :, 2 * BC : 3 * BC].transpose(0, 2, 1).reshape(B, S)
    label = np.mean((dmin_i.sum(1) + dmin_j.sum(1)) / S)

    # BCE (host, f32, matching the reference's formulas)
    l0 = logits[:, :, 0].astype(f)
    e0 = np.exp(l0).astype(f)
    rs = (1.0 / s).astype(f)
    p0 = (e0 * rs).astype(f)
    logp = np.maximum((l0 - np.log(s).astype(f)).astype(f), f(-100.0))
    lom = np.maximum(np.log1p(-p0).astype(f), f(-100.0))
    et = (mh == 0)                                     # eos_target
    bce = np.where(et, -logp, -lom).astype(f)
    ep = (t == EOS).astype(f)
    eh = mh
    cep, ceh = ep.sum(1), eh.sum(1)
    eos = np.mean(
        0.5 * (bce * ep).sum(1) / (cep + EPS)
        + 0.5 * (bce * eh).sum(1) / (ceh + EPS)
    )
    return np.stack([label, eos]).astype(f)


def kernel(logits, targets):
    logits, lgt16, eg, mh, t = _prep(logits, targets)
    maps = _in_maps(lgt16, eg, mh, logits)
    nc = _get_nc()

    if os.environ.get("KMODE") == "sim":
        from concourse import bass_interp

        outs = []
        for c in range(M):
            sim = bass_interp.CoreSim(nc)
            for k, v in maps[c].items():
                sim.tensor(k)[:] = v
            sim.simulate()
            outs.append(np.array(sim.tensor("out")))
    else:
        import time

        from concourse.bass_utils import run_bass_kernel_spmd

        # the axon terminal occasionally reports a transient mesh desync;
        # a short backoff and retry recovers it
        last_err = None
        for attempt in range(3):
            try:
                res = run_bass_kernel_spmd(nc, maps, list(range(M)))
                break
            except Exception as e:  # noqa: BLE001
                last_err = e
                time.sleep(30 * (attempt + 1))
        else:
            raise last_err
        outs = [res.results[c]["out"] for c in range(M)]

    return _combine(outs, logits, mh, t)
